# revision 3
# baseline (speedup 1.0000x reference)
"""Dilated attention Bass kernel for 8 Trainium2 NeuronCores — v3.

Same numerics as the baseline (fp16 hi/lo q/k/V, fp16 exp'd scores), with:
  - chunk-pair processing: K=64 correction MMs of adjacent chunks target
    disjoint PE row groups and run concurrently (measured dstart ~3ns)
  - PV stationary operands zero-padded to 128 columns so their LDWEIGHTS
    pull ahead of in-flight matmuls (65-col loads serialize, +104ns/MM)
  - reciprocal_approx_fast for the 1/l row (DVE reciprocal was 5.2us each)

Per sub-problem on-device (fp32 PSUM accumulation):
  S^T[k,q] = khi.T(qhi+qlo) + klo.T qhi    (K=128 stacked MM + row-packed
                                            64x128 correction MM)
  E        = exp(S^T) in fp16              (softmax scale folded into q)
  O'[d,q]  = [Vhi|1].T E + [Vlo|0].T E     (row 64 = softmax denominator l)
  x        = O'[0:64] * (1/l)              (recip row broadcast via DRAM DMA)
  out      = x / (4 * sum_{segs,q} x)      per (head, channel)
"""

import os
import numpy as np

import concourse.bass as bass
import concourse.bacc as bacc
import concourse.mybir as mybir
import concourse.tile as tile
from concourse import bass_utils

# ---------------------------------------------------------------- constants
B, S, H, D = 2, 8192, 16, 64
SEGMENT_LENGTHS = [1024, 2048, 4096, 8192]
DILATION_RATES = [1, 2, 4, 8]
NUM_GROUPS = 4
GROUP_HEADS = H // NUM_GROUPS  # 4
SEGS_PER_GROUP = [S // s for s in SEGMENT_LENGTHS]  # [8, 4, 2, 1]
NPROB = sum(SEGS_PER_GROUP)  # 15 problems per core
SL = 1024          # per-problem sequence length
NCHUNK = SL // 128  # 8 key chunks
N_CORES = 8
SCALE = 1.0 / np.sqrt(D)

BF16 = mybir.dt.bfloat16
FP32 = mybir.dt.float32
FP16 = mybir.dt.float16
VW = D + 1  # 65: V plus the ones column


def _problem_list(j):
    """15 (group, head, seg) tuples for local head-slot j, head-contiguous."""
    out = []
    for g in range(NUM_GROUPS):
        head = g * GROUP_HEADS + j
        for seg in range(SEGS_PER_GROUP[g]):
            out.append((g, head, seg))
    return out


def _positions(g, seg):
    s, r = SEGMENT_LENGTHS[g], DILATION_RATES[g]
    offset = g % r
    return seg * s + offset + r * np.arange(SL)


def _mm(nc, noldw=False, **kw):
    return nc.tensor.matmul(**kw)


# ---------------------------------------------------------------- device IR
def _build_tile_program(ctx, tc, out_ap, qd_ap, kk_ap, vp_ap):
    nc = tc.nc
    EXP = mybir.ActivationFunctionType.Exp

    qd_pool = ctx.enter_context(tc.tile_pool(name="qd", bufs=3))
    kk_pool = ctx.enter_context(tc.tile_pool(name="kk", bufs=3))
    vp_pool = ctx.enter_context(tc.tile_pool(name="vp", bufs=3))
    exp_pool = ctx.enter_context(tc.tile_pool(name="exps", bufs=4))
    sout_pool = ctx.enter_context(tc.tile_pool(name="sout", bufs=3))
    snorm_pool = ctx.enter_context(tc.tile_pool(name="snorm", bufs=11))
    rrow_pool = ctx.enter_context(tc.tile_pool(name="rrow", bufs=3))
    sums_pool = ctx.enter_context(tc.tile_pool(name="sums", bufs=6))
    fin_pool = ctx.enter_context(tc.tile_pool(name="fin", bufs=3))
    rlb_pool = ctx.enter_context(tc.tile_pool(name="rlb", bufs=3))
    rdram_pool = ctx.enter_context(
        tc.tile_pool(name="rdram", bufs=2, space="DRAM"))
    spsum = ctx.enter_context(tc.tile_pool(name="spsum", bufs=3, space="PSUM"))
    pvpsum = ctx.enter_context(tc.tile_pool(name="pvpsum", bufs=1, space="PSUM"))

    ones_t = None  # [128, 64] fp32 ones, for the tail matmul-broadcast

    # per-problem state; problems are head-contiguous
    probs = []
    for g in range(NUM_GROUPS):
        for seg in range(SEGS_PER_GROUP[g]):
            probs.append({
                "first": seg == 0,
                "last": seg == SEGS_PER_GROUP[g] - 1,
            })
    for p, st in enumerate(probs):
        st["p"] = p
    head_lists = []
    i = 0
    for nseg in SEGS_PER_GROUP:
        head_lists.append(probs[i:i + nseg])
        i += nseg
    for hl in head_lists:
        for st in hl:
            st["head_list"] = hl

    def emit_bcast_mm(st, pv_ps):
        # tail-only: broadcast l via ones.T @ l matmul into pv rows 0-63
        # (PE is idle at the kernel tail; avoids the ~6us DRAM round trip)
        for h in range(2):
            nc.tensor.matmul(
                out=pv_ps[0:D, h * 512:(h + 1) * 512],
                lhsT=ones_t[64:65, :],
                rhs=st["s_out"][D:D + 1, h * 512:(h + 1) * 512],
                start=True, stop=True)
        st["rl_b"] = pv_ps[0:D, :]

    def emit_bcast(st):
        # broadcast the l row to 64 partitions: SBUF -> DRAM -> stride-0 DMA.
        # (reciprocal happens after the broadcast: reciprocal_approx_fast
        # mishandles non-zero base partitions, so it must run at partition 0)
        r_d = rdram_pool.tile([1, SL], FP32)
        nc.gpsimd.dma_start(out=r_d, in_=st["s_out"][D:D + 1, :])
        rl_b = rlb_pool.tile([D, SL], FP32)
        st["rl_b"] = rl_b
        src = bass.AP(tensor=r_d.tensor, offset=r_d.offset,
                      ap=[[0, D]] + [list(d) for d in r_d.ap[1:]])
        nc.gpsimd.dma_start(out=rl_b, in_=src)

    def emit_norm(st):
        # s_norm = s_out[0:64] * recip(bcast(l)); seg_sum = sum_q s_norm + prev
        prev_accum = None if st["first"] else probs[st["p"] - 1]["seg_sum"]
        rinv = rlb_pool.tile([D, SL], FP32, tag="rinv")
        nc.vector.reciprocal_approx_fast(out=rinv, in_=st["rl_b"])
        s_norm = snorm_pool.tile([D, SL], FP32)
        seg_local = sums_pool.tile([D, 1], FP32, tag="seg_local")
        nc.vector.tensor_mul(s_norm, st["s_out"][0:D, :], rinv)
        nc.vector.reduce_sum(seg_local, s_norm, axis=mybir.AxisListType.X)
        if prev_accum is None:
            seg_sum = seg_local
        else:
            seg_sum = sums_pool.tile([D, 1], FP32, tag="seg_sum")
            nc.vector.tensor_add(seg_sum, seg_local, prev_accum)
        st["s_norm"] = s_norm
        st["seg_sum"] = seg_sum
        if st["last"]:
            emit_head_finals(st)

    def emit_head_finals(last_st):
        # rh = 1 / (4 * head_sum); out = s_norm * rh, DMA out
        hs4 = sums_pool.tile([D, 1], FP32)
        nc.vector.tensor_scalar_mul(hs4, last_st["seg_sum"], float(NUM_GROUPS))
        rh = sums_pool.tile([D, 1], FP32)
        nc.vector.reciprocal(out=rh, in_=hs4)
        for st in last_st["head_list"]:
            fin = fin_pool.tile([D, SL], FP16)
            nc.vector.tensor_scalar_mul(fin, st["s_norm"], rh)
            nc.gpsimd.dma_start(out=out_ap[st["p"]], in_=fin)

    ones_t = sums_pool.tile([128, D], FP32, tag="ones")
    nc.vector.memset(ones_t, 1.0)

    for p in range(NPROB):
        st = probs[p]

        qd_t = qd_pool.tile([128, 2 * SL], FP16)
        nc.sync.dma_start(out=qd_t, in_=qd_ap[p])
        kk_t = kk_pool.tile([128, SL], FP16)
        nc.sync.dma_start(out=kk_t, in_=kk_ap[p])
        vp_t = vp_pool.tile([128, NCHUNK * 256], FP16)
        nc.sync.dma_start(out=vp_t, in_=vp_ap[p])

        pv_ps = pvpsum.tile([128, SL], FP32, tag="pv")
        for c0 in range(0, NCHUNK, 2):
            pair = (c0, c0 + 1)
            s_tiles = {}
            for c in pair:
                s_t = spsum.tile([128, SL], FP32, tag="s")
                s_tiles[c] = s_t

            # stacked K=128 MMs: (khi+klo).T @ qhi (kk holds khi and klo in
            # opposite partition halves; rhs is qhi duplicated in both halves)
            for c in pair:
                for h in range(2):
                    _mm(nc,
                        out=s_tiles[c][:, h * 512:(h + 1) * 512],
                        lhsT=kk_t[:, c * 128:(c + 1) * 128],
                        rhs=qd_t[:, h * 512:(h + 1) * 512],
                        start=True, stop=False)

            # K=64 corrections: khi.T qlo; adjacent MMs alternate row groups
            # (c even -> rows 0-63, c odd -> rows 64-127) and run concurrently
            for h in range(2):
                for c in pair:
                    base = (c % 2) * 64
                    _mm(nc,
                        out=s_tiles[c][:, h * 512:(h + 1) * 512],
                        lhsT=kk_t[base:base + 64, c * 128:(c + 1) * 128],
                        rhs=qd_t[base:base + 64, SL + h * 512: SL + (h + 1) * 512],
                        start=False, stop=True)

            e_tiles = {}
            for c in pair:
                e_t = exp_pool.tile([128, SL], FP16, tag="e")
                nc.scalar.activation(out=e_t, in_=s_tiles[c], func=EXP)
                e_tiles[c] = e_t

            # PV: [Vhi|1|0].T E + [Vlo|0].T E with 128-col (zero-padded)
            # stationary operands so LDWEIGHTS pulls ahead of in-flight MMs
            for c in pair:
                for lohi in range(2):
                    ws = slice(c * 256 + lohi * 128, c * 256 + (lohi + 1) * 128)
                    for h in range(2):
                        _mm(nc,
                            out=pv_ps[:, h * 512:(h + 1) * 512],
                            lhsT=vp_t[:, ws],
                            rhs=e_tiles[c][:, h * 512:(h + 1) * 512],
                            start=(c == 0 and lohi == 0),
                            stop=(c == NCHUNK - 1 and lohi == 1))

        # epilogue: evacuate PV psum, 1/l row, bcast, normalize, seg sums
        s_out = sout_pool.tile([VW, SL], FP32)
        nc.vector.tensor_copy(out=s_out, in_=pv_ps[0:VW, :])
        st["s_out"] = s_out
        if p == NPROB - 1:
            emit_bcast_mm(st, pv_ps)
        else:
            emit_bcast(st)
        emit_norm(st)


# Cache: the Bass program is identical for every call (and every core).
_CACHED = {}


def _get_program():
    key = "v4"
    if key in _CACHED:
        return _CACHED[key]
    nc = bacc.Bacc("TRN2", target_bir_lowering=False, debug=False)
    qd = nc.dram_tensor("qd", [NPROB, 128, 2 * SL], FP16,
                        kind="ExternalInput").ap()
    kk = nc.dram_tensor("kk", [NPROB, 128, SL], FP16,
                        kind="ExternalInput").ap()
    vp = nc.dram_tensor("vp", [NPROB, 128, NCHUNK * 256], FP16,
                        kind="ExternalInput").ap()
    out = nc.dram_tensor("out", [NPROB, D, SL], FP16, kind="ExternalOutput").ap()
    from contextlib import ExitStack
    with tile.TileContext(nc) as tc, ExitStack() as ctx:
        _build_tile_program(ctx, tc, out, qd, kk, vp)
    nc.compile()
    _CACHED[key] = nc
    return nc


# ---------------------------------------------------------------- host glue
def _prep_core(q, k, v, b, j):
    """Build the qka/qkb/vp device inputs for core (b, j). q is pre-scaled."""
    f16 = np.float16
    qd = np.empty((NPROB, 128, 2 * SL), dtype=f16)
    kk = np.empty((NPROB, 128, SL), dtype=f16)
    vp = np.zeros((NPROB, 128, NCHUNK * 256), dtype=f16)
    ones = np.ones((SL, 1), np.float32)
    for p, (g, head, seg) in enumerate(_problem_list(j)):
        pos = _positions(g, seg)
        qT = q[b, pos, head, :].T  # [64, 1024] fp32, already scaled
        kT = k[b, pos, head, :].T
        qhi = qT.astype(f16)
        qlo = (qT - qhi.astype(np.float32)).astype(f16)
        khi = kT.astype(f16)
        klo = (kT - khi.astype(np.float32)).astype(f16)
        # qd: cols 0-1023 qhi duplicated, cols 1024-2047 qlo duplicated
        qd[p, 0:64, 0:SL] = qhi
        qd[p, 64:128, 0:SL] = qhi
        qd[p, 0:64, SL:] = qlo
        qd[p, 64:128, SL:] = qlo
        # kk: per chunk, khi in the (c%2) partition half and klo in the other
        for c in range(NCHUNK):
            cs = slice(c * 128, (c + 1) * 128)
            base = (c % 2) * 64
            kk[p, base:base + 64, cs] = khi[:, cs]
            kk[p, 64 - base:128 - base, cs] = klo[:, cs]
        vs = v[b, pos, head, :]  # [1024, 64] fp32
        vhi = vs.astype(f16)
        vlo = (vs - vhi.astype(np.float32)).astype(f16)
        vfull = np.zeros((SL, 256), np.float32)
        vfull[:, 0:D] = vhi.astype(np.float32)
        vfull[:, D:D + 1] = ones
        vfull[:, 128:128 + D] = vlo.astype(np.float32)
        vp[p] = (vfull.reshape(NCHUNK, 128, 256)
                 .transpose(1, 0, 2).reshape(128, NCHUNK * 256)
                 .astype(f16))
    return {"qd": qd, "kk": kk, "vp": vp}


def kernel(query, key, value, _run_kw=None):
    q = np.asarray(query, dtype=np.float32)
    k = np.asarray(key, dtype=np.float32)
    v = np.asarray(value, dtype=np.float32)
    qs = q * SCALE  # fold softmax scale into q

    nc = _get_program()
    in_maps = []
    core_meta = []
    for core in range(N_CORES):
        b, j = divmod(core, NUM_GROUPS)
        in_maps.append(_prep_core(qs, k, v, b, j))
        core_meta.append((b, j))

    kw = dict(_run_kw or {})
    kw.pop("result", None)
    res = bass_utils.run_bass_kernel_spmd(
        nc, in_maps, core_ids=list(range(N_CORES)), **kw)

    out = np.zeros((B, S, H, D), dtype=np.float32)
    for core in range(N_CORES):
        b, j = core_meta[core]
        dev_out = res.results[core]["out"]  # [15, 64, 1024] fp32
        for p, (g, head, seg) in enumerate(_problem_list(j)):
            pos = _positions(g, seg)
            out[b, pos, head, :] = dev_out[p].T
    if _run_kw is not None:
        _run_kw["result"] = res
    return out


# revision 4
# speedup vs baseline: 1.0010x; 1.0010x over previous
"""Dilated attention Bass kernel for 8 Trainium2 NeuronCores — v3.

Same numerics as the baseline (fp16 hi/lo q/k/V, fp16 exp'd scores), with:
  - chunk-pair processing: K=64 correction MMs of adjacent chunks target
    disjoint PE row groups and run concurrently (measured dstart ~3ns)
  - PV stationary operands zero-padded to 128 columns so their LDWEIGHTS
    pull ahead of in-flight matmuls (65-col loads serialize, +104ns/MM)
  - reciprocal_approx_fast for the 1/l row (DVE reciprocal was 5.2us each)

Per sub-problem on-device (fp32 PSUM accumulation):
  S^T[k,q] = khi.T(qhi+qlo) + klo.T qhi    (K=128 stacked MM + row-packed
                                            64x128 correction MM)
  E        = exp(S^T) in fp16              (softmax scale folded into q)
  O'[d,q]  = [Vhi|1].T E + [Vlo|0].T E     (row 64 = softmax denominator l)
  x        = O'[0:64] * (1/l)              (recip row broadcast via DRAM DMA)
  out      = x / (4 * sum_{segs,q} x)      per (head, channel)
"""

import os
import numpy as np

import concourse.bass as bass
import concourse.bacc as bacc
import concourse.mybir as mybir
import concourse.tile as tile
from concourse import bass_utils

# ---------------------------------------------------------------- constants
B, S, H, D = 2, 8192, 16, 64
SEGMENT_LENGTHS = [1024, 2048, 4096, 8192]
DILATION_RATES = [1, 2, 4, 8]
NUM_GROUPS = 4
GROUP_HEADS = H // NUM_GROUPS  # 4
SEGS_PER_GROUP = [S // s for s in SEGMENT_LENGTHS]  # [8, 4, 2, 1]
NPROB = sum(SEGS_PER_GROUP)  # 15 problems per core
SL = 1024          # per-problem sequence length
NCHUNK = SL // 128  # 8 key chunks
N_CORES = 8
SCALE = 1.0 / np.sqrt(D)

BF16 = mybir.dt.bfloat16
FP32 = mybir.dt.float32
FP16 = mybir.dt.float16
VW = D + 1  # 65: V plus the ones column


def _problem_list(j):
    """15 (group, head, seg) tuples for local head-slot j, head-contiguous."""
    out = []
    for g in range(NUM_GROUPS):
        head = g * GROUP_HEADS + j
        for seg in range(SEGS_PER_GROUP[g]):
            out.append((g, head, seg))
    return out


def _positions(g, seg):
    s, r = SEGMENT_LENGTHS[g], DILATION_RATES[g]
    offset = g % r
    return seg * s + offset + r * np.arange(SL)


def _mm(nc, noldw=False, **kw):
    return nc.tensor.matmul(**kw)


# ---------------------------------------------------------------- device IR
def _build_tile_program(ctx, tc, out_ap, qd_ap, kk_ap, vp_ap):
    nc = tc.nc
    EXP = mybir.ActivationFunctionType.Exp

    qd_pool = ctx.enter_context(tc.tile_pool(name="qd", bufs=3))
    kk_pool = ctx.enter_context(tc.tile_pool(name="kk", bufs=3))
    vp_pool = ctx.enter_context(tc.tile_pool(name="vp", bufs=3))
    exp_pool = ctx.enter_context(tc.tile_pool(name="exps", bufs=4))
    sout_pool = ctx.enter_context(tc.tile_pool(name="sout", bufs=3))
    snorm_pool = ctx.enter_context(tc.tile_pool(name="snorm", bufs=11))
    rrow_pool = ctx.enter_context(tc.tile_pool(name="rrow", bufs=3))
    sums_pool = ctx.enter_context(tc.tile_pool(name="sums", bufs=6))
    fin_pool = ctx.enter_context(tc.tile_pool(name="fin", bufs=3))
    rlb_pool = ctx.enter_context(tc.tile_pool(name="rlb", bufs=3))
    rdram_pool = ctx.enter_context(
        tc.tile_pool(name="rdram", bufs=2, space="DRAM"))
    spsum = ctx.enter_context(tc.tile_pool(name="spsum", bufs=3, space="PSUM"))
    pvpsum = ctx.enter_context(tc.tile_pool(name="pvpsum", bufs=1, space="PSUM"))

    ones_t = None  # [128, 64] fp32 ones, for the tail matmul-broadcast

    # per-problem state; problems are head-contiguous
    probs = []
    for g in range(NUM_GROUPS):
        for seg in range(SEGS_PER_GROUP[g]):
            probs.append({
                "first": seg == 0,
                "last": seg == SEGS_PER_GROUP[g] - 1,
            })
    for p, st in enumerate(probs):
        st["p"] = p
    head_lists = []
    i = 0
    for nseg in SEGS_PER_GROUP:
        head_lists.append(probs[i:i + nseg])
        i += nseg
    for hl in head_lists:
        for st in hl:
            st["head_list"] = hl

    def emit_bcast_mm(st, pv_ps):
        # tail-only: broadcast l via ones.T @ l matmul into pv rows 0-63
        # (PE is idle at the kernel tail; avoids the ~6us DRAM round trip)
        for h in range(2):
            nc.tensor.matmul(
                out=pv_ps[0:D, h * 512:(h + 1) * 512],
                lhsT=ones_t[64:65, :],
                rhs=st["s_out"][D:D + 1, h * 512:(h + 1) * 512],
                start=True, stop=True)
        st["rl_b"] = pv_ps[0:D, :]

    def emit_bcast(st):
        # broadcast the l row to 64 partitions: SBUF -> DRAM -> stride-0 DMA.
        # (reciprocal happens after the broadcast: reciprocal_approx_fast
        # mishandles non-zero base partitions, so it must run at partition 0)
        r_d = rdram_pool.tile([1, SL], FP32)
        nc.gpsimd.dma_start(out=r_d, in_=st["s_out"][D:D + 1, :])
        rl_b = rlb_pool.tile([D, SL], FP32)
        st["rl_b"] = rl_b
        src = bass.AP(tensor=r_d.tensor, offset=r_d.offset,
                      ap=[[0, D]] + [list(d) for d in r_d.ap[1:]])
        nc.gpsimd.dma_start(out=rl_b, in_=src)

    def emit_norm(st):
        # s_norm = s_out[0:64] * recip(bcast(l)); seg_sum = sum_q s_norm + prev
        prev_accum = None if st["first"] else probs[st["p"] - 1]["seg_sum"]
        rinv = rlb_pool.tile([D, SL], FP32, tag="rinv")
        nc.vector.reciprocal_approx_fast(out=rinv, in_=st["rl_b"])
        s_norm = snorm_pool.tile([D, SL], FP32)
        seg_local = sums_pool.tile([D, 1], FP32, tag="seg_local")
        nc.vector.tensor_mul(s_norm, st["s_out"][0:D, :], rinv)
        nc.vector.reduce_sum(seg_local, s_norm, axis=mybir.AxisListType.X)
        if prev_accum is None:
            seg_sum = seg_local
        else:
            seg_sum = sums_pool.tile([D, 1], FP32, tag="seg_sum")
            nc.vector.tensor_add(seg_sum, seg_local, prev_accum)
        st["s_norm"] = s_norm
        st["seg_sum"] = seg_sum
        if st["last"]:
            emit_head_finals(st)

    def emit_head_finals(last_st):
        # rh = 1 / (4 * head_sum); out = s_norm * rh, DMA out
        hs4 = sums_pool.tile([D, 1], FP32)
        nc.vector.tensor_scalar_mul(hs4, last_st["seg_sum"], float(NUM_GROUPS))
        rh = sums_pool.tile([D, 1], FP32)
        nc.vector.reciprocal(out=rh, in_=hs4)
        for st in last_st["head_list"]:
            fin = fin_pool.tile([D, SL], FP16)
            nc.vector.tensor_scalar_mul(fin, st["s_norm"], rh)
            nc.gpsimd.dma_start(out=out_ap[st["p"]], in_=fin)

    ones_t = sums_pool.tile([128, D], FP32, tag="ones")
    nc.vector.memset(ones_t, 1.0)

    for p in range(NPROB):
        st = probs[p]

        qd_t = qd_pool.tile([128, 2 * SL], FP16)
        nc.sync.dma_start(out=qd_t[0:64, :], in_=qd_ap[p])
        nc.sync.dma_start(out=qd_t[64:128, :], in_=qd_t[0:64, :])
        kk_t = kk_pool.tile([128, SL], FP16)
        nc.sync.dma_start(out=kk_t, in_=kk_ap[p])
        vp_t = vp_pool.tile([128, NCHUNK * 256], FP16)
        vp_dst = bass.AP(tensor=vp_t.tensor, offset=vp_t.offset,
                         ap=[[NCHUNK * 256, 128], [128, 2 * NCHUNK], [1, VW]])
        vp_src = bass.AP(tensor=vp_ap.tensor,
                         offset=vp_ap[p].offset,
                         ap=[[2 * NCHUNK * VW, 128], [VW, 2 * NCHUNK], [1, VW]])
        nc.sync.dma_start(out=vp_dst, in_=vp_src)

        pv_ps = pvpsum.tile([128, SL], FP32, tag="pv")
        for c0 in range(0, NCHUNK, 2):
            pair = (c0, c0 + 1)
            s_tiles = {}
            for c in pair:
                s_t = spsum.tile([128, SL], FP32, tag="s")
                s_tiles[c] = s_t

            # stacked K=128 MMs: (khi+klo).T @ qhi (kk holds khi and klo in
            # opposite partition halves; rhs is qhi duplicated in both halves)
            for c in pair:
                for h in range(2):
                    _mm(nc,
                        out=s_tiles[c][:, h * 512:(h + 1) * 512],
                        lhsT=kk_t[:, c * 128:(c + 1) * 128],
                        rhs=qd_t[:, h * 512:(h + 1) * 512],
                        start=True, stop=False)

            # K=64 corrections: khi.T qlo; adjacent MMs alternate row groups
            # (c even -> rows 0-63, c odd -> rows 64-127) and run concurrently
            for h in range(2):
                for c in pair:
                    base = (c % 2) * 64
                    _mm(nc,
                        out=s_tiles[c][:, h * 512:(h + 1) * 512],
                        lhsT=kk_t[base:base + 64, c * 128:(c + 1) * 128],
                        rhs=qd_t[base:base + 64, SL + h * 512: SL + (h + 1) * 512],
                        start=False, stop=True)

            e_tiles = {}
            for c in pair:
                e_t = exp_pool.tile([128, SL], FP16, tag="e")
                nc.scalar.activation(out=e_t, in_=s_tiles[c], func=EXP)
                e_tiles[c] = e_t

            # PV: [Vhi|1|0].T E + [Vlo|0].T E with 128-col (zero-padded)
            # stationary operands so LDWEIGHTS pulls ahead of in-flight MMs
            for c in pair:
                for lohi in range(2):
                    ws = slice(c * 256 + lohi * 128, c * 256 + (lohi + 1) * 128)
                    for h in range(2):
                        _mm(nc,
                            out=pv_ps[:, h * 512:(h + 1) * 512],
                            lhsT=vp_t[:, ws],
                            rhs=e_tiles[c][:, h * 512:(h + 1) * 512],
                            start=(c == 0 and lohi == 0),
                            stop=(c == NCHUNK - 1 and lohi == 1))

        # epilogue: evacuate PV psum, 1/l row, bcast, normalize, seg sums
        s_out = sout_pool.tile([VW, SL], FP32)
        nc.vector.tensor_copy(out=s_out, in_=pv_ps[0:VW, :])
        st["s_out"] = s_out
        if p == NPROB - 1:
            emit_bcast_mm(st, pv_ps)
        else:
            emit_bcast(st)
        emit_norm(st)


# Cache: the Bass program is identical for every call (and every core).
_CACHED = {}


def _get_program():
    key = "v5"
    if key in _CACHED:
        return _CACHED[key]
    nc = bacc.Bacc("TRN2", target_bir_lowering=False, debug=False)
    qd = nc.dram_tensor("qd", [NPROB, 64, 2 * SL], FP16,
                        kind="ExternalInput").ap()
    kk = nc.dram_tensor("kk", [NPROB, 128, SL], FP16,
                        kind="ExternalInput").ap()
    vp = nc.dram_tensor("vp", [NPROB, 128, 2 * NCHUNK * VW], FP16,
                        kind="ExternalInput").ap()
    out = nc.dram_tensor("out", [NPROB, D, SL], FP16, kind="ExternalOutput").ap()
    from contextlib import ExitStack
    with tile.TileContext(nc) as tc, ExitStack() as ctx:
        _build_tile_program(ctx, tc, out, qd, kk, vp)
    nc.compile()
    _CACHED[key] = nc
    return nc


# ---------------------------------------------------------------- host glue
def _prep_core(q, k, v, b, j):
    """Build the qka/qkb/vp device inputs for core (b, j). q is pre-scaled."""
    f16 = np.float16
    qd = np.empty((NPROB, 64, 2 * SL), dtype=f16)
    kk = np.empty((NPROB, 128, SL), dtype=f16)
    vp = np.zeros((NPROB, 128, 2 * NCHUNK * VW), dtype=f16)
    ones = np.ones((SL, 1), np.float32)
    for p, (g, head, seg) in enumerate(_problem_list(j)):
        pos = _positions(g, seg)
        qT = q[b, pos, head, :].T  # [64, 1024] fp32, already scaled
        kT = k[b, pos, head, :].T
        qhi = qT.astype(f16)
        qlo = (qT - qhi.astype(np.float32)).astype(f16)
        khi = kT.astype(f16)
        klo = (kT - khi.astype(np.float32)).astype(f16)
        # qd: cols 0-1023 qhi, cols 1024-2047 qlo (device duplicates rows)
        qd[p, :, 0:SL] = qhi
        qd[p, :, SL:] = qlo
        # kk: per chunk, khi in the (c%2) partition half and klo in the other
        for c in range(NCHUNK):
            cs = slice(c * 128, (c + 1) * 128)
            base = (c % 2) * 64
            kk[p, base:base + 64, cs] = khi[:, cs]
            kk[p, 64 - base:128 - base, cs] = klo[:, cs]
        vs = v[b, pos, head, :]  # [1024, 64] fp32
        vhi = vs.astype(f16)
        vlo = (vs - vhi.astype(np.float32)).astype(f16)
        vfull = np.zeros((SL, 2 * VW), np.float32)
        vfull[:, 0:D] = vhi.astype(np.float32)
        vfull[:, D:D + 1] = ones
        vfull[:, VW:VW + D] = vlo.astype(np.float32)
        vp[p] = (vfull.reshape(NCHUNK, 128, 2 * VW)
                 .transpose(1, 0, 2).reshape(128, NCHUNK * 2 * VW)
                 .astype(f16))
    return {"qd": qd, "kk": kk, "vp": vp}


def kernel(query, key, value, _run_kw=None):
    q = np.asarray(query, dtype=np.float32)
    k = np.asarray(key, dtype=np.float32)
    v = np.asarray(value, dtype=np.float32)
    qs = q * SCALE  # fold softmax scale into q

    nc = _get_program()
    in_maps = []
    core_meta = []
    for core in range(N_CORES):
        b, j = divmod(core, NUM_GROUPS)
        in_maps.append(_prep_core(qs, k, v, b, j))
        core_meta.append((b, j))

    kw = dict(_run_kw or {})
    kw.pop("result", None)
    res = bass_utils.run_bass_kernel_spmd(
        nc, in_maps, core_ids=list(range(N_CORES)), **kw)

    out = np.zeros((B, S, H, D), dtype=np.float32)
    for core in range(N_CORES):
        b, j = core_meta[core]
        dev_out = res.results[core]["out"]  # [15, 64, 1024] fp32
        for p, (g, head, seg) in enumerate(_problem_list(j)):
            pos = _positions(g, seg)
            out[b, pos, head, :] = dev_out[p].T
    if _run_kw is not None:
        _run_kw["result"] = res
    return out
